# revision 1
# baseline (speedup 1.0000x reference)
"""Trainium2 Bass kernel for nn_ItemEmbeddingLayer (fused double-gather + concat).

Strategy: vocab-parallel across 8 NeuronCores. Core c owns vocab shard
[c*12544, (c+1)*12544). Host routes each index to its owning core (sharding),
cores build a 768B-padded fused table shard on-device (emb||genre||pad), then
dma_gather their assigned rows and write them out; host un-shards by placing
each returned row at its original batch position.
"""
import sys

sys.path.insert(0, "/opt/trn_rl_repo")
import numpy as np

import concourse.bacc as bacc
import concourse.tile as tile
from concourse import mybir
from concourse.bass_utils import run_bass_kernel_spmd

P = 128
D, Dg = 128, 18
F = 192            # padded fused row: 146 f32 -> 192 f32 (768B, %256)
VSH = 12544        # vocab rows per core shard (98*128); 8*12544 >= 100000
NV = VSH // P      # 98 build iterations of 128 rows
R2 = 1024          # rows gathered per dma_gather call
NCH = 132          # chunks per core -> capacity 135168 rows/core
CAPC = NCH * R2
W16 = R2 // 16     # 256

_nc_cache = {}


def _build_nc():
    nc = bacc.Bacc(None, target_bir_lowering=False, debug=False)
    f32, i16 = mybir.dt.float32, mybir.dt.int16
    idx_t = nc.dram_tensor("idx", [NCH, 16, W16], i16, kind="ExternalInput")
    emb_t = nc.dram_tensor("embsh", [VSH, D], f32, kind="ExternalInput")
    gen_t = nc.dram_tensor("gensh", [VSH, Dg], f32, kind="ExternalInput")
    out_t = nc.dram_tensor("out", [NCH, P, R2 // P, F], f32, kind="ExternalOutput")
    fsh_t = nc.dram_tensor("fsh", [VSH, F], f32)  # internal padded fused shard
    with tile.TileContext(nc) as tc:
        with (
            tc.tile_pool(name="build", bufs=4) as bpool,
            tc.tile_pool(name="idxp", bufs=3) as ipool,
            tc.tile_pool(name="rows", bufs=3) as rpool,
        ):
            # ---- build fused padded shard table via SBUF bounce ----
            for v in range(NV):
                bt = bpool.tile([P, F], f32)
                nc.vector.memset(bt[:], 0.0)
                nc.sync.dma_start(out=bt[:, 0:D], in_=emb_t.ap()[v * P:(v + 1) * P, :])
                nc.scalar.dma_start(out=bt[:, D:D + Dg], in_=gen_t.ap()[v * P:(v + 1) * P, :])
                nc.sync.dma_start(out=fsh_t.ap()[v * P:(v + 1) * P, :], in_=bt[:])
            # ---- gather loop ----
            for ch in range(NCH):
                it = ipool.tile([P, W16], i16)
                for g in range(8):
                    nc.sync.dma_start(out=it[16 * g:16 * (g + 1), :], in_=idx_t.ap()[ch])
                rt = rpool.tile([P, R2 // P, F], f32)
                nc.gpsimd.dma_gather(
                    out_ap=rt[:],
                    in_ap=fsh_t.ap(),
                    idxs_ap=it[:],
                    num_idxs=R2,
                    num_idxs_reg=R2,
                    elem_size=F,
                )
                nc.sync.dma_start(out=out_t.ap()[ch], in_=rt[:])
    nc.compile()
    return nc


def kernel(item_inputs, item_embedding, genre_table):
    B = item_inputs.shape[0]
    idx = np.asarray(item_inputs).astype(np.int64)
    emb = np.ascontiguousarray(np.asarray(item_embedding, dtype=np.float32))
    gen = np.ascontiguousarray(np.asarray(genre_table, dtype=np.float32))
    V = emb.shape[0]

    if "nc" not in _nc_cache:
        _nc_cache["nc"] = _build_nc()
    nc = _nc_cache["nc"]

    # ---- host-side sharding: route each index to its owning core ----
    shard = (idx // VSH).astype(np.int64)
    in_maps, positions, lens = [], [], []
    for c in range(8):
        pos_c = np.nonzero(shard == c)[0]
        loc_c = (idx[pos_c] - c * VSH).astype(np.int16)
        n = len(loc_c)
        assert n <= CAPC, f"shard {c} overflow: {n} > {CAPC}"
        lens.append(n)
        positions.append(pos_c)
        loc_pad = np.zeros(CAPC, np.int16)
        loc_pad[:n] = loc_c
        # wrap-16 layout per chunk: list position k=f*16+q -> [ch, q, f]
        idx_w = loc_pad.reshape(NCH, W16, 16).transpose(0, 2, 1).copy()
        # per-core vocab shard slices (zero-pad the tail shard)
        lo, hi = c * VSH, min((c + 1) * VSH, V)
        esh = np.zeros((VSH, D), np.float32)
        gsh = np.zeros((VSH, Dg), np.float32)
        esh[: hi - lo] = emb[lo:hi]
        gsh[: hi - lo] = gen[lo:hi]
        in_maps.append({"idx": idx_w, "embsh": esh, "gensh": gsh})

    _nc_cache["in_maps"] = in_maps
    res = run_bass_kernel_spmd(nc, in_maps, core_ids=list(range(8)))

    # ---- host-side unshard: place rows back at original positions ----
    out = np.empty((B, D + Dg), np.float32)
    for c in range(8):
        o = res.results[c]["out"][:, :, :, : D + Dg]
        rows = o.transpose(0, 2, 1, 3).reshape(CAPC, D + Dg)
        out[positions[c]] = rows[: lens[c]]
    return out



# revision 4
# speedup vs baseline: 2.9129x; 2.9129x over previous
"""Trainium2 Bass kernel for nn_ItemEmbeddingLayer (fused double-gather + concat).

Strategy: vocab-parallel across 8 NeuronCores. Core c owns vocab shard
[c*12544, (c+1)*12544). The host builds ONE fused bf16 table per shard
(emb||genre padded to 512B rows, exactly the dma_gather 256B-multiple
minimum at full descriptor rate), routes each index to its owning core,
and permutes the per-core int16 index list so the device's natural
output-row order equals sorted-batch order. Each core dma_gathers its
rows (512B each, no on-device table build), compacts them to 146-wide
on the Vector engine (so the out-DMA is one multi-KB descriptor per
partition instead of 292B strided pieces), and writes bf16 rows out.
Host does a single fused cast+scatter back to original batch positions.
bf16 keeps rel err <= 2^-9, well inside the 2e-2 gate, and halves both
DMA time and PJRT wire bytes vs the padded-f32 design. Index loads are
grouped K chunks per DMA from a host-replicated layout - per-chunk
replication DMAs were the serial bottleneck on the SP engine.
"""
import sys

sys.path.insert(0, "/opt/trn_rl_repo")
import numpy as np
import ml_dtypes

import concourse.bacc as bacc
import concourse.tile as tile
from concourse import mybir
from concourse.bass_utils import run_bass_kernel_spmd

BF16 = np.dtype(ml_dtypes.bfloat16)

P = 128
D, Dg = 128, 18
DF = D + Dg        # 146 real columns per fused row
E = 256            # padded fused row: 146 bf16 -> 256 bf16 (512B, %256)
VSH = 12544        # vocab rows per core shard; 8*12544 >= 100000
R2 = 1024          # rows gathered per dma_gather call (SWDGE ring-safe)
NCH = 132          # chunks per core -> capacity 135168 rows/core
K = 12             # chunks per grouped idx load
NG = NCH // K      # 11 idx-load groups
CAPC = NCH * R2
W16 = R2 // 16     # idx wrap width
NR = R2 // P       # 8 gathered rows per partition per chunk

_nc_cache = {}


def _build_nc():
    nc = bacc.Bacc(None, target_bir_lowering=False, debug=False)
    bf16, i16 = mybir.dt.bfloat16, mybir.dt.int16
    idx_t = nc.dram_tensor("idx", [NG, P, K * W16], i16, kind="ExternalInput")
    tab_t = nc.dram_tensor("tab", [VSH, E], bf16, kind="ExternalInput")
    out_t = nc.dram_tensor("out", [NCH, P, NR, DF], bf16, kind="ExternalOutput")
    with tile.TileContext(nc) as tc:
        with (
            tc.tile_pool(name="idxp", bufs=2) as ipool,
            tc.tile_pool(name="rows", bufs=3) as rpool,
            tc.tile_pool(name="cmp", bufs=3) as cpool,
        ):
            for g in range(NG):
                it = ipool.tile([P, K * W16], i16)
                nc.sync.dma_start(out=it[:], in_=idx_t.ap()[g])
                for k in range(K):
                    ch = g * K + k
                    rt = rpool.tile([P, NR, E], bf16)
                    nc.gpsimd.dma_gather(
                        out_ap=rt[:],
                        in_ap=tab_t.ap(),
                        idxs_ap=it[:, k * W16:(k + 1) * W16],
                        num_idxs=R2,
                        num_idxs_reg=R2,
                        elem_size=E,
                    )
                    ct = cpool.tile([P, NR, DF], bf16)
                    nc.vector.tensor_copy(out=ct[:], in_=rt[:, :, 0:DF])
                    nc.scalar.dma_start(out=out_t.ap()[ch], in_=ct[:])
    nc.compile()
    return nc


def _devperm():
    # device flat row j = [ch, p, c2] was fed from idx list position
    # k = ch*R2 + c2*128 + p; jperm[k] = j inverts that so feeding
    # loc_sorted[jperm] makes device row j hold sorted-batch element j.
    k = np.arange(CAPC)
    r = k % R2
    return (k - r) + (r % P) * NR + r // P


def kernel(item_inputs, item_embedding, genre_table):
    idx = np.asarray(item_inputs).astype(np.int64)
    emb = np.asarray(item_embedding, dtype=np.float32)
    gen = np.asarray(genre_table, dtype=np.float32)
    B, V = idx.shape[0], emb.shape[0]

    if "nc" not in _nc_cache:
        _nc_cache["nc"] = _build_nc()
    nc = _nc_cache["nc"]

    # ---- host-side sharding: route each index to its owning core ----
    shard = idx // VSH
    order = np.argsort(shard, kind="stable")
    counts = np.bincount(shard, minlength=8)
    loc_sorted = (idx[order] - shard[order] * VSH).astype(np.int16)
    jperm = _devperm()

    # fused bf16 table: [emb || genre || pad] per row, 512B rows
    tab = np.zeros((8 * VSH, E), BF16)
    tab[:V, :D] = emb.astype(BF16)
    tab[:V, D:DF] = gen.astype(BF16)

    in_maps, positions = [], []
    off = 0
    for c in range(8):
        n = int(counts[c])
        assert n <= CAPC, f"shard {c} overflow: {n} > {CAPC}"
        positions.append(order[off:off + n])
        loc_pad = np.zeros(CAPC, np.int16)
        loc_pad[:n] = loc_sorted[off:off + n]
        off += n
        # place element j at list position k (loc_list[k] = loc_pad[jperm[k]]),
        # wrap-16 per chunk (position k = f*16 + q -> [ch, q, f]), replicate
        # across the 8 gpsimd cores, and group K chunks per idx-load DMA.
        idx_w = loc_pad[jperm].reshape(NCH, W16, 16).transpose(0, 2, 1)
        idx_r = np.broadcast_to(idx_w.reshape(NG, K, 1, 16, W16), (NG, K, 8, 16, W16))
        idx_g = idx_r.transpose(0, 2, 3, 1, 4).reshape(NG, P, K * W16).copy()
        in_maps.append({"idx": idx_g, "tab": tab[c * VSH:(c + 1) * VSH]})

    _nc_cache["in_maps"] = in_maps
    res = run_bass_kernel_spmd(nc, in_maps, core_ids=list(range(8)))

    # ---- host-side unshard: device row j is sorted-batch element j ----
    out = np.empty((B, DF), np.float32)
    for c in range(8):
        rows = res.results[c]["out"].reshape(CAPC, DF)
        out[positions[c]] = rows[: len(positions[c])]  # fused bf16->f32 cast + scatter
    return out
